# revision 5
# baseline (speedup 1.0000x reference)
"""GATv2 layer (N=50000, D=128, H=4, E=600000) on 8 trn2 NeuronCores — v2.

Architecture (vs v1 baseline):
- Degree-aligned edge layout: dst nodes are sorted by in-degree per core and
  assigned to 128-row windows; partition row p of window w holds ALL edges of
  one dst node, padded to the window max degree Kw.  This makes the segment
  softmax and the weighted aggregation per-partition reductions — no
  selection-matrix matmuls, no per-edge xr gather.
- bf16 xl table + dma_gather (InstDMAGatherAnt, int16 idx) instead of
  row-at-a-time indirect DMA: one gather op covers a whole window (128*Kw
  rows).  int16 range is handled with a lo/hi split view of one table plus
  dedicated zero rows (exactly one of the two gathers returns nonzero and
  they are summed on DVE).
- Phase 1 (xl = h@W_l) uses host-cast bf16 h loaded via HWDGE DMA-transpose,
  bf16 matmuls, bf16 table stores.
- Softmax without max-subtraction (scores are O(+-10)); bias_out dropped
  (absorbed exactly by BatchNorm mean subtraction); normalization folded to
  a per-node divide after aggregation.  BN stats AllReduced across cores.
"""

import math
import numpy as np
import ml_dtypes

import concourse.bass as bass
import concourse.bacc as bacc
import concourse.mybir as mybir
import concourse.tile as tile
from concourse.masks import make_identity
from concourse.bass_utils import run_bass_kernel_spmd

P = 128
F32 = mybir.dt.float32
BF16 = mybir.dt.bfloat16
I16 = mybir.dt.int16
I32 = mybir.dt.int32

NEG_SLOPE = 0.2
BN_EPS = 1e-5
NPBF16 = ml_dtypes.bfloat16


class Cfg:
    def __init__(self, N, D, H, E, n_cores, KW, locap=32767, hibase=None,
                 tgroup=2048, has_bl=False, has_br=False):
        assert D == 128
        self.N, self.D, self.H, self.E = N, D, H, E
        self.C = D // H
        self.n_cores = n_cores
        assert N % n_cores == 0
        self.NPC = N // n_cores
        self.W = math.ceil(self.NPC / P)
        self.LASTR = self.NPC - P * (self.W - 1)
        self.KW = list(KW)                       # per-window max degree
        assert len(self.KW) == self.W
        self.KMAX = max(self.KW)
        self.SUMKW = sum(self.KW)
        self.TGROUP = tgroup                     # rows per phase-1 group
        self.HPAD = math.ceil((N + 2) / tgroup) * tgroup
        self.NG = self.HPAD // tgroup
        self.TAB = self.HPAD + 2                 # xl table rows (xl at +1)
        self.LOCAP = locap                       # lo view: src <= locap-1
        self.SENT = N + 1                        # hi zero row (table pos)
        if hibase is None:
            hibase = max(0, self.SENT - locap)
        self.HIBASE = hibase
        assert self.SENT - hibase <= locap
        assert locap + 1 <= self.TAB
        self.HVIEW = min(locap + 1, self.TAB - hibase)  # hi view rows
        assert hibase + self.HVIEW >= self.SENT + 1
        self.TAB2 = self.TAB // 2                # pair rows (512B each)
        assert (self.N + 1) // 2 + 1 < 32767
        self.HAS_BL = has_bl
        self.HAS_BR = has_br
        # local node rows padded to full windows
        self.LPAD = self.W * P
        self.LGROUPS = []                        # phase-1b transpose groups
        r = 0
        while r < self.LPAD:
            g = min(tgroup, self.LPAD - r)
            g = (g // 16) * 16
            self.LGROUPS.append((r, g))
            r += g
        assert r == self.LPAD


def build_kernel(nc: bass.Bass, cfg: Cfg):
    W, H, C, KW = cfg.W, cfg.H, cfg.C, cfg.KW
    KMAX, SUMKW = cfg.KMAX, cfg.SUMKW
    TG, NG = cfg.TGROUP, cfg.NG
    LASTR = cfg.LASTR
    N = cfg.N

    # ---- I/O ----
    h_b = nc.declare_dram_parameter("h_b", [cfg.HPAD, P], BF16, isOutput=False)
    hloc_b = nc.declare_dram_parameter("hloc_b", [cfg.LPAD, P], BF16,
                                       isOutput=False)
    hres = nc.declare_dram_parameter("hres", [cfg.LPAD, P], F32,
                                     isOutput=False)
    idxp = nc.declare_dram_parameter("idxp", [P, 8 * SUMKW], I16,
                                      isOutput=False)
    maskp = nc.declare_dram_parameter("maskp", [P, SUMKW], BF16,
                                      isOutput=False)
    parp = nc.declare_dram_parameter("parp", [P, SUMKW], BF16,
                                     isOutput=False)
    constsA = nc.declare_dram_parameter("constsA", [P, 3], F32, isOutput=False)
    constsB = nc.declare_dram_parameter("constsB", [P, 2 * P], BF16,
                                        isOutput=False)
    constsC = nc.declare_dram_parameter("constsC", [64, P], BF16,
                                        isOutput=False)
    out = nc.declare_dram_parameter("out", [cfg.NPC, P], F32, isOutput=True)

    # ---- internal DRAM ----
    tab = nc.dram_tensor("tab", [cfg.TAB, P], BF16)
    st_in = nc.dram_tensor("st_in", [P, 2], F32)
    st_out = nc.dram_tensor("st_out", [P, 2], F32, addr_space="Shared")

    # pair view: row t = [xl-table-row 2t | row 2t+1], 512B per row
    tab_pair = tab[:].rearrange("(t b) f -> t (b f)", b=2)

    with tile.TileContext(nc) as tc:
        import contextlib
        with contextlib.ExitStack() as ctx:
            cst = ctx.enter_context(tc.tile_pool(name="cst", bufs=1))
            sb = ctx.enter_context(tc.tile_pool(name="sb", bufs=2))
            pG = ctx.enter_context(tc.tile_pool(name="pG", bufs=2))
            pZ = ctx.enter_context(tc.tile_pool(name="pZ", bufs=2))
            ps = ctx.enter_context(tc.tile_pool(name="ps", bufs=2,
                                                space="PSUM"))
            ps1 = ctx.enter_context(tc.tile_pool(name="ps1", bufs=1,
                                                 space="PSUM"))
            ps2 = ctx.enter_context(tc.tile_pool(name="ps2", bufs=1,
                                                 space="PSUM"))

            # ================= constants =================
            csA = cst.tile([P, 3], F32, tag="csA")
            nc.sync.dma_start(out=csA[:], in_=constsA[:])
            att_col = csA[:, 0:1]
            gam_col = csA[:, 1:2]
            bet_col = csA[:, 2:3]
            csB = cst.tile([P, 2 * P], BF16, tag="csB")
            nc.sync.dma_start(out=csB[:], in_=constsB[:])
            Wl_bf = csB[:, 0:P]
            Wr_bf = csB[:, P:2 * P]
            csC = cst.tile([64, P], BF16, tag="csC")
            nc.sync.dma_start(out=csC[:], in_=constsC[:])
            bl_row = csC[0:1, :]
            br_row = csC[32:33, :]

            ones1_bf = cst.tile([64, P], BF16, tag="ones1")
            nc.gpsimd.memset(ones1_bf[:], 1.0)
            ones_col = cst.tile([P, 1], F32, tag="ones_c")
            nc.gpsimd.memset(ones_col[:], 1.0)
            zero_row = cst.tile([1, P], BF16, tag="zrow")
            nc.gpsimd.memset(zero_row[:], 0.0)
            ident = cst.tile([P, P], F32, tag="ident")
            make_identity(nc, ident[:])
            eps_col = cst.tile([P, 1], F32, tag="epsc")
            nc.gpsimd.memset(eps_col[:], BN_EPS)
            ones_msk = cst.tile([P, 1], F32, tag="ones_m")
            if LASTR < P:
                pidx = cst.tile([P, 1], I32, tag="pidx")
                nc.gpsimd.iota(pidx[:], pattern=[[0, 1]], channel_multiplier=1)
                nc.vector.tensor_scalar(out=ones_msk[:], in0=pidx[:],
                                        scalar1=LASTR, scalar2=None,
                                        op0=mybir.AluOpType.is_lt)
            else:
                nc.gpsimd.memset(ones_msk[:], 1.0)

            # att replicated to all partitions (bf16)
            att_ps = ps.tile([P, P], F32, tag="ptr")
            nc.tensor.transpose(att_ps[:], att_col.to_broadcast([P, P]),
                                ident[:])
            att_rep = cst.tile([P, P], BF16, tag="attrep")
            nc.scalar.copy(att_rep[:], att_ps[:])

            # pinned phase-2 operands
            idx_pin = cst.tile([P, 8 * SUMKW], I16, tag="idx")
            nc.sync.dma_start(out=idx_pin[:], in_=idxp[:])
            msk_pin = cst.tile([P, SUMKW], BF16, tag="msk")
            nc.sync.dma_start(out=msk_pin[:], in_=maskp[:])
            par_pin = cst.tile([P, SUMKW], BF16, tag="par")
            nc.sync.dma_start(out=par_pin[:], in_=parp[:])
            xr_pin = cst.tile([P, cfg.LPAD], BF16, tag="xrp")
            outpre = cst.tile([P, cfg.LPAD], F32, tag="opre")

            # zero the lo-view sentinel row (tab[0]); hi sentinel (tab[SENT])
            # is written by phase 1 from zero-padded h rows, but write it
            # explicitly in case of nonzero bias_l.
            nc.sync.dma_start(out=tab[0:1, :], in_=zero_row[:])
            nc.sync.dma_start(out=tab[cfg.SENT:cfg.SENT + 1, :],
                              in_=zero_row[:])
            nc.sync.dma_start(out=tab[cfg.TAB - 1:cfg.TAB, :],
                              in_=zero_row[:])

            # ================= phase 1: xl table =================
            NT = TG // P  # tiles per group
            for g in range(NG):
                hT = sb.tile([P, TG], BF16, tag="hT")
                nc.sync.dma_start(out=hT[:], in_=h_b[g * TG:(g + 1) * TG, :],
                                  transpose=True)
                xt = sb.tile([P, TG], BF16, tag="xt")
                for jj in range(NT // 4):
                    pst = ps.tile([P, 4 * P], F32, tag="p1")
                    for j in range(4):
                        t = jj * 4 + j
                        nc.tensor.matmul(pst[:, j * P:(j + 1) * P],
                                         lhsT=hT[:, t * P:(t + 1) * P],
                                         rhs=Wl_bf, start=True,
                                         stop=not cfg.HAS_BL)
                        if cfg.HAS_BL:
                            nc.tensor.matmul(pst[:, j * P:(j + 1) * P],
                                             lhsT=ones1_bf[0:1, :], rhs=bl_row,
                                             start=False, stop=True)
                    nc.scalar.copy(xt[:, jj * 4 * P:(jj + 1) * 4 * P],
                                   pst[:])
                # store rows [g*TG+1, (g+1)*TG+1) — xl row j at tab[j+1]
                nc.scalar.dma_start(
                    out=tab[g * TG + 1:(g + 1) * TG + 1, :].rearrange(
                        "(a p) f -> p a f", p=P),
                    in_=xt[:].rearrange("p (a f) -> p a f", f=P))

            # ================= phase 1b: xr (local, stays in SBUF) ========
            for (r0, rows) in cfg.LGROUPS:
                hTl = sb.tile([P, TG], BF16, tag="hTl")
                nc.sync.dma_start(out=hTl[:, :rows],
                                  in_=hloc_b[r0:r0 + rows, :], transpose=True)
                nt = rows // P
                for jj in range(math.ceil(nt / 4)):
                    cols = min(4, nt - jj * 4)
                    pst = ps.tile([P, 4 * P], F32, tag="p1")
                    for j in range(cols):
                        t = jj * 4 + j
                        nc.tensor.matmul(pst[:, j * P:(j + 1) * P],
                                         lhsT=hTl[:, t * P:(t + 1) * P],
                                         rhs=Wr_bf, start=True,
                                         stop=not cfg.HAS_BR)
                        if cfg.HAS_BR:
                            nc.tensor.matmul(pst[:, j * P:(j + 1) * P],
                                             lhsT=ones1_bf[32:33, :],
                                             rhs=br_row,
                                             start=False, stop=True)
                    c0 = r0 + jj * 4 * P
                    nc.scalar.copy(
                        xr_pin[:, c0:c0 + cols * P], pst[:, :cols * P])

            tc.strict_bb_all_engine_barrier()

            # ================= phase 2: edges =================
            # group adjacent windows into one dma_gather (cap NI<=128*GCAP)
            GCAP = 36
            groups = []
            gstart = 0
            while gstart < W:
                kg = 0
                wend = gstart
                while wend < W and kg + KW[wend] <= GCAP:
                    kg += KW[wend]
                    wend += 1
                groups.append((gstart, wend, kg))
                gstart = wend
            stats_ps = ps1.tile([P, 1], F32, tag="stats")
            stats2_ps = ps2.tile([P, 1], F32, tag="stats2")
            koffs = [0]
            for w in range(W):
                koffs.append(koffs[-1] + KW[w])
            for (w0, w1, Kg) in groups:
                kg0 = koffs[w0]
                NI = P * Kg
                g2 = pG.tile([P, GCAP, 2 * P], BF16, tag="G2")
                nc.gpsimd.dma_gather(
                    g2[:, :Kg, :], tab_pair,
                    idx_pin[:, 8 * kg0:8 * (kg0 + Kg)],
                    num_idxs=NI, num_idxs_reg=NI, elem_size=2 * P,
                    elem_step=2 * P, single_packet=False)
                # y = parity-select(g2); then add xr per member window
                y = pG.tile([P, GCAP, P], BF16, tag="y")
                nc.vector.tensor_tensor(out=y[:, :Kg, :],
                                        in0=g2[:, :Kg, P:2 * P],
                                        in1=g2[:, :Kg, 0:P],
                                        op=mybir.AluOpType.subtract)
                nc.vector.tensor_tensor(
                    out=y[:, :Kg, :], in0=y[:, :Kg, :],
                    in1=par_pin[:, kg0:kg0 + Kg][:, :, None]
                        .to_broadcast([P, Kg, P]),
                    op=mybir.AluOpType.mult)
                nc.vector.tensor_tensor(out=y[:, :Kg, :], in0=y[:, :Kg, :],
                                        in1=g2[:, :Kg, 0:P],
                                        op=mybir.AluOpType.add)
                for w in range(w0, w1):
                    rel = koffs[w] - kg0
                    Kw = KW[w]
                    nc.vector.tensor_tensor(
                        out=y[:, rel:rel + Kw, :], in0=y[:, rel:rel + Kw, :],
                        in1=xr_pin[:, None, w * P:(w + 1) * P]
                            .to_broadcast([P, Kw, P]),
                        op=mybir.AluOpType.add)
                # z = LeakyReLU(y); zz = z*att; s = sum_c zz; alpha=exp*mask
                z = pZ.tile([P, GCAP, P], BF16, tag="z")
                nc.scalar.activation(z[:, :Kg, :], y[:, :Kg, :],
                                     mybir.ActivationFunctionType.Prelu,
                                     alpha=NEG_SLOPE)
                w2 = pZ.tile([P, GCAP, P], BF16, tag="w2")
                nc.vector.tensor_tensor(
                    out=w2[:, :Kg, :], in0=z[:, :Kg, :],
                    in1=att_rep[:, None, :].to_broadcast([P, Kg, P]),
                    op=mybir.AluOpType.mult)
                s = pZ.tile([P, GCAP, H], F32, tag="s")
                nc.vector.tensor_reduce(
                    out=s[:, :Kg, :][:, :, :, None],
                    in_=w2[:, :Kg, :].rearrange("p k (h c) -> p k h c", c=C),
                    op=mybir.AluOpType.add, axis=mybir.AxisListType.X)
                al = pZ.tile([P, GCAP, H], BF16, tag="al")
                nc.scalar.activation(al[:, :Kg, :], s[:, :Kg, :],
                                     mybir.ActivationFunctionType.Exp)
                nc.vector.tensor_tensor(
                    out=al[:, :Kg, :], in0=al[:, :Kg, :],
                    in1=msk_pin[:, kg0:kg0 + Kg][:, :, None]
                        .to_broadcast([P, Kg, H]),
                    op=mybir.AluOpType.mult)
                nc.vector.tensor_tensor(
                    out=w2[:, :Kg, :].rearrange("p k (h c) -> p k h c", c=C),
                    in0=y[:, :Kg, :].rearrange("p k (h c) -> p k h c", c=C),
                    in1=al[:, :Kg, :][:, :, :, None]
                        .to_broadcast([P, Kg, H, C]),
                    op=mybir.AluOpType.mult)
                # per-window: reductions + normalize + BN stats
                for w in range(w0, w1):
                    rel = koffs[w] - kg0
                    Kw = KW[w]
                    acc = pZ.tile([P, P], F32, tag="acc")
                    nc.vector.tensor_reduce(
                        out=acc[:][:, :, None],
                        in_=w2[:, rel:rel + Kw, :].rearrange("p k f -> p f k"),
                        op=mybir.AluOpType.add, axis=mybir.AxisListType.X)
                    den = pZ.tile([P, H], F32, tag="den")
                    nc.vector.tensor_reduce(
                        out=den[:][:, :, None],
                        in_=al[:, rel:rel + Kw, :].rearrange("p k h -> p h k"),
                        op=mybir.AluOpType.add, axis=mybir.AxisListType.X)
                    rec = pZ.tile([P, H], F32, tag="rec")
                    nc.vector.reciprocal(rec[:], den[:])
                    op_sl = outpre[:, w * P:(w + 1) * P]
                    nc.vector.tensor_tensor(
                        out=op_sl.rearrange("p (h c) -> p h c", c=C),
                        in0=acc[:].rearrange("p (h c) -> p h c", c=C),
                        in1=rec[:, :, None].to_broadcast([P, H, C]),
                        op=mybir.AluOpType.mult)
                    nc.vector.tensor_tensor(
                        out=op_sl, in0=op_sl,
                        in1=xr_pin[:, w * P:(w + 1) * P],
                        op=mybir.AluOpType.subtract)
                    sq = pZ.tile([P, P], F32, tag="sq")
                    nc.scalar.square(sq[:], op_sl)
                    stat_ones = ones_msk if w == W - 1 else ones_col
                    nc.tensor.matmul(stats_ps[:], lhsT=op_sl,
                                     rhs=stat_ones[:],
                                     start=(w == 0), stop=(w == W - 1))
                    nc.tensor.matmul(stats2_ps[:], lhsT=sq[:],
                                     rhs=stat_ones[:],
                                     start=(w == 0), stop=(w == W - 1))

            # ================= phase 3: BN stats AllReduce =================
            st_sb = sb.tile([P, 2], F32, tag="stsb")
            nc.scalar.copy(st_sb[:, 0:1], stats_ps[:])
            nc.scalar.copy(st_sb[:, 1:2], stats2_ps[:])
            nc.sync.dma_start(out=st_in[:], in_=st_sb[:])
            tc.strict_bb_all_engine_barrier()
            nc.gpsimd.collective_compute(
                "AllReduce", mybir.AluOpType.add,
                replica_groups=[list(range(cfg.n_cores))],
                ins=[st_in[:]], outs=[st_out[:]])
            tc.strict_bb_all_engine_barrier()
            st_all = sb.tile([P, 2], F32, tag="stall")
            nc.sync.dma_start(out=st_all[:], in_=st_out[:])

            # A = gamma * rsqrt(var+eps); B = beta - A*mu  (y = A*x + B)
            mu_c = sb.tile([P, 1], F32, tag="mu")
            nc.scalar.mul(mu_c[:], st_all[:, 0:1], 1.0 / N)
            ex2 = sb.tile([P, 1], F32, tag="ex2")
            nc.scalar.mul(ex2[:], st_all[:, 1:2], 1.0 / N)
            mu2 = sb.tile([P, 1], F32, tag="mu2")
            nc.scalar.square(mu2[:], mu_c[:])
            var_c = sb.tile([P, 1], F32, tag="var")
            nc.vector.tensor_tensor(out=var_c[:], in0=ex2[:], in1=mu2[:],
                                    op=mybir.AluOpType.subtract)
            sd = sb.tile([P, 1], F32, tag="sd")
            nc.scalar.activation(sd[:], var_c[:],
                                 mybir.ActivationFunctionType.Sqrt,
                                 bias=eps_col[:])
            rsd = sb.tile([P, 1], F32, tag="rsd")
            nc.vector.reciprocal(rsd[:], sd[:])
            A_c = sb.tile([P, 1], F32, tag="Ac")
            nc.vector.tensor_tensor(out=A_c[:], in0=gam_col, in1=rsd[:],
                                    op=mybir.AluOpType.mult)
            Amu = sb.tile([P, 1], F32, tag="Amu")
            nc.vector.tensor_tensor(out=Amu[:], in0=A_c[:], in1=mu_c[:],
                                    op=mybir.AluOpType.mult)
            B_c = sb.tile([P, 1], F32, tag="Bc")
            nc.vector.tensor_tensor(out=B_c[:], in0=bet_col, in1=Amu[:],
                                    op=mybir.AluOpType.subtract)

            A_ps = ps.tile([P, P], F32, tag="ptr")
            nc.tensor.transpose(A_ps[:], A_c[:].to_broadcast([P, P]), ident[:])
            A_rep = cst.tile([P, P], F32, tag="Arep")
            nc.scalar.copy(A_rep[:], A_ps[:])
            B_ps = ps.tile([P, P], F32, tag="ptr")
            nc.tensor.transpose(B_ps[:], B_c[:].to_broadcast([P, P]), ident[:])
            B_rep = cst.tile([P, P], F32, tag="Brep")
            nc.scalar.copy(B_rep[:], B_ps[:])

            # ================= phase 4: BN apply + relu + residual ==========
            # batched 4 windows per op; the final (partial) window separate
            FB = 4
            wfull = W - 1
            wb = 0
            while wb < wfull:
                nb = min(FB, wfull - wb)
                cols = nb * P
                c0 = wb * P
                t1 = sb.tile([P, FB * P], F32, tag="t1")
                nc.vector.tensor_tensor(
                    out=t1[:, :cols].rearrange("p (a f) -> p a f", f=P),
                    in0=outpre[:, c0:c0 + cols].rearrange(
                        "p (a f) -> p a f", f=P),
                    in1=A_rep[:, None, :].to_broadcast([P, nb, P]),
                    op=mybir.AluOpType.mult)
                t2 = sb.tile([P, FB * P], F32, tag="t2")
                nc.vector.tensor_tensor(
                    out=t2[:, :cols].rearrange("p (a f) -> p a f", f=P),
                    in0=t1[:, :cols].rearrange("p (a f) -> p a f", f=P),
                    in1=B_rep[:, None, :].to_broadcast([P, nb, P]),
                    op=mybir.AluOpType.add)
                r = sb.tile([P, FB * P], F32, tag="r")
                nc.scalar.activation(r[:, :cols], t2[:, :cols],
                                     mybir.ActivationFunctionType.Relu)
                hr = sb.tile([P, FB * P], F32, tag="hr")
                nc.sync.dma_start(
                    out=hr[:, :cols].rearrange("p (a f) -> p a f", f=P),
                    in_=hres[c0:c0 + cols, :].rearrange(
                        "(a p) f -> p a f", p=P))
                o = sb.tile([P, FB * P], F32, tag="o")
                nc.vector.tensor_tensor(out=o[:, :cols], in0=r[:, :cols],
                                        in1=hr[:, :cols],
                                        op=mybir.AluOpType.add)
                nc.sync.dma_start(
                    out=out[c0:c0 + cols, :].rearrange("(a p) f -> p a f",
                                                       p=P),
                    in_=o[:, :cols].rearrange("p (a f) -> p a f", f=P))
                wb += nb
            # final window (partial rows)
            w = W - 1
            t1 = sb.tile([P, P], F32, tag="t1l")
            nc.vector.tensor_tensor(out=t1[:], in0=outpre[:, w * P:(w + 1) * P],
                                    in1=A_rep[:], op=mybir.AluOpType.mult)
            t2 = sb.tile([P, P], F32, tag="t2l")
            nc.vector.tensor_tensor(out=t2[:], in0=t1[:], in1=B_rep[:],
                                    op=mybir.AluOpType.add)
            r = sb.tile([P, P], F32, tag="rl")
            nc.scalar.activation(r[:], t2[:],
                                 mybir.ActivationFunctionType.Relu)
            hr = sb.tile([P, P], F32, tag="hrl")
            nc.sync.dma_start(out=hr[:], in_=hres[w * P:(w + 1) * P, :])
            o = sb.tile([P, P], F32, tag="ol")
            nc.vector.tensor_tensor(out=o[:], in0=r[:], in1=hr[:],
                                    op=mybir.AluOpType.add)
            nc.sync.dma_start(out=out[w * P:w * P + LASTR, :],
                              in_=o[:LASTR, :])
    return nc


def host_prepare(h, edge_index, W_l, W_r, bias_l, bias_r, att,
                 bias_out, gamma, beta, n_cores=8):
    N, D = h.shape
    H, C = att.shape
    E = edge_index.shape[1]
    h = np.asarray(h, np.float32)
    ei = np.asarray(edge_index)

    loops = np.arange(N, dtype=np.int64)
    src = np.concatenate([ei[0], loops]).astype(np.int64)
    dst = np.concatenate([ei[1], loops]).astype(np.int64)
    order = np.argsort(dst, kind="stable")
    src_s = src[order].astype(np.int32)
    dst_s = dst[order].astype(np.int32)

    NPC = N // n_cores
    W = math.ceil(NPC / P)
    LPAD = W * P
    bounds = np.searchsorted(dst_s, np.arange(0, N + 1, NPC))

    # per-core degree sort
    percore = []
    kw_all = np.zeros((n_cores, W), np.int64)
    for k in range(n_cores):
        lo, hi = bounds[k], bounds[k + 1]
        s_k = src_s[lo:hi]
        d_k = dst_s[lo:hi] - k * NPC
        deg = np.bincount(d_k, minlength=NPC)
        perm = np.argsort(-deg, kind="stable")   # node order, high-deg first
        dsort = deg[perm]
        dpad = np.concatenate([dsort, np.zeros(LPAD - NPC, np.int64)])
        kw_all[k] = dpad.reshape(W, P).max(axis=1)
        starts = np.concatenate([[0], np.cumsum(deg)])
        percore.append((s_k, deg, perm, starts))
    KW = kw_all.max(axis=0)
    KW = np.maximum(KW, 1)

    cfg = Cfg(N=N, D=D, H=H, E=E, n_cores=n_cores, KW=KW,
              has_bl=bool(np.any(np.asarray(bias_l))),
              has_br=bool(np.any(np.asarray(bias_r))))
    SUMKW = cfg.SUMKW

    # shared tensors
    h_b = np.zeros((cfg.HPAD, P), NPBF16)
    h_b[:N] = h.astype(NPBF16)
    constsA = np.zeros((P, 3), np.float32)
    constsA[:, 0] = np.asarray(att, np.float32).reshape(-1)
    constsA[:, 1] = np.asarray(gamma, np.float32)
    constsA[:, 2] = np.asarray(beta, np.float32)
    constsB = np.zeros((P, 2 * P), np.float32)
    constsB[:, 0:P] = np.asarray(W_l, np.float32)
    constsB[:, P:2 * P] = np.asarray(W_r, np.float32)
    constsB = constsB.astype(NPBF16)
    constsC = np.zeros((64, P), np.float32)
    constsC[0] = np.asarray(bias_l, np.float32)
    constsC[32] = np.asarray(bias_r, np.float32)
    constsC = constsC.astype(NPBF16)

    in_maps = []
    perms = []
    for k in range(n_cores):
        s_k, deg, perm, starts = percore[k]
        perms.append(perm)
        # slot tables [LPAD rows, KW[w] cols per window]
        idx = np.zeros((P, 8 * SUMKW), np.int16)
        msk = np.zeros((P, SUMKW), np.float32)
        par = np.zeros((P, SUMKW), np.float32)
        koff = 0
        for w in range(W):
            Kw = int(KW[w])
            NI = P * Kw
            # srcs[p, j] = j-th edge's src of node perm[w*128+p]
            i16 = np.zeros(NI, np.int32)
            for p in range(P):
                v = w * P + p
                if v >= NPC:
                    # pad row: slot 0 stays idx 0 / par 0 (-> zero half),
                    # unmask it so den >= 1 and no divide-by-zero
                    msk[p, koff] = 1.0
                    continue
                node = perm[v]
                e0, e1 = starts[node], starts[node + 1]
                d = e1 - e0
                if d == 0:
                    msk[p, koff] = 1.0
                    continue
                srcs = s_k[e0:e1].astype(np.int64)
                pos = np.arange(d) * P + p      # slot i = k*128+p
                t = srcs + 1                    # xl[src] at table row src+1
                i16[pos] = t >> 1
                par[p, koff:koff + d] = (t & 1).astype(np.float32)
                msk[p, koff:koff + d] = 1.0
            # wrap: [16, NI/16] then replicate to 128 partitions
            blk = i16.reshape(-1, 16).T.astype(np.int16)
            idx[:, 8 * koff:8 * (koff + Kw)] = np.tile(blk, (8, 1))
            koff += Kw

        gidx = k * NPC + perm                   # global ids, perm order
        hloc_b = np.zeros((LPAD, P), NPBF16)
        hloc_b[:NPC] = h[gidx].astype(NPBF16)
        hres = np.zeros((LPAD, P), np.float32)
        hres[:NPC] = h[gidx]

        in_maps.append({
            "h_b": h_b, "hloc_b": hloc_b, "hres": hres,
            "idxp": idx,
            "maskp": msk.astype(NPBF16),
            "parp": par.astype(NPBF16),
            "constsA": constsA, "constsB": constsB, "constsC": constsC,
        })
    return cfg, in_maps, perms


def kernel(h, edge_index, W_l, W_r, bias_l, bias_r, att,
           bias_out, gamma, beta):
    n_cores = 8
    cfg, in_maps, perms = host_prepare(
        h, edge_index, W_l, W_r, bias_l, bias_r, att, bias_out, gamma, beta,
        n_cores=n_cores)
    nc = bacc.Bacc()
    build_kernel(nc, cfg)
    nc.compile()
    res = run_bass_kernel_spmd(nc, in_maps, core_ids=list(range(n_cores)))
    N = cfg.N
    out_full = np.empty((N, cfg.D), np.float32)
    for k in range(n_cores):
        gidx = k * cfg.NPC + perms[k]
        out_full[gidx] = res.results[k]["out"]
    return out_full


# revision 7
# speedup vs baseline: 1.1736x; 1.1736x over previous
"""GATv2 layer (N=50000, D=128, H=4, E=600000) on 8 trn2 NeuronCores — v2.

Architecture (vs v1 baseline):
- Degree-aligned edge layout: dst nodes are sorted by in-degree per core and
  assigned to 128-row windows; partition row p of window w holds ALL edges of
  one dst node, padded to the window max degree Kw.  This makes the segment
  softmax and the weighted aggregation per-partition reductions — no
  selection-matrix matmuls, no per-edge xr gather.
- bf16 xl table + dma_gather (InstDMAGatherAnt, int16 idx) instead of
  row-at-a-time indirect DMA: one gather op covers a whole window (128*Kw
  rows).  int16 range is handled with a lo/hi split view of one table plus
  dedicated zero rows (exactly one of the two gathers returns nonzero and
  they are summed on DVE).
- Phase 1 (xl = h@W_l) uses host-cast bf16 h loaded via HWDGE DMA-transpose,
  bf16 matmuls, bf16 table stores.
- Softmax without max-subtraction (scores are O(+-10)); bias_out dropped
  (absorbed exactly by BatchNorm mean subtraction); normalization folded to
  a per-node divide after aggregation.  BN stats AllReduced across cores.
"""

import math
import numpy as np
import ml_dtypes

import concourse.bass as bass
import concourse.bacc as bacc
import concourse.mybir as mybir
import concourse.tile as tile
from concourse.masks import make_identity
from concourse.bass_utils import run_bass_kernel_spmd

P = 128
F32 = mybir.dt.float32
BF16 = mybir.dt.bfloat16
I16 = mybir.dt.int16
I32 = mybir.dt.int32

NEG_SLOPE = 0.2
BN_EPS = 1e-5
NPBF16 = ml_dtypes.bfloat16


class Cfg:
    def __init__(self, N, D, H, E, n_cores, KW, locap=32767, hibase=None,
                 tgroup=2048, has_bl=False, has_br=False):
        assert D == 128
        self.N, self.D, self.H, self.E = N, D, H, E
        self.C = D // H
        self.n_cores = n_cores
        assert N % n_cores == 0
        self.NPC = N // n_cores
        self.W = math.ceil(self.NPC / P)
        self.LASTR = self.NPC - P * (self.W - 1)
        self.KW = list(KW)                       # per-window max degree
        assert len(self.KW) == self.W
        self.KMAX = max(self.KW)
        self.SUMKW = sum(self.KW)
        self.TGROUP = tgroup                     # rows per phase-1 group
        self.HPAD = math.ceil((N + 2) / tgroup) * tgroup
        self.NG = self.HPAD // tgroup
        self.TAB = self.HPAD + 2                 # xl table rows (xl at +1)
        self.LOCAP = locap                       # lo view: src <= locap-1
        self.SENT = N + 1                        # hi zero row (table pos)
        if hibase is None:
            hibase = max(0, self.SENT - locap)
        self.HIBASE = hibase
        assert self.SENT - hibase <= locap
        assert locap + 1 <= self.TAB
        self.HVIEW = min(locap + 1, self.TAB - hibase)  # hi view rows
        assert hibase + self.HVIEW >= self.SENT + 1
        self.TAB2 = self.TAB // 2                # pair rows (512B each)
        assert (self.N + 1) // 2 + 1 < 32767
        self.HAS_BL = has_bl
        self.HAS_BR = has_br
        # local node rows padded to full windows
        self.LPAD = self.W * P
        self.LGROUPS = []                        # phase-1b transpose groups
        r = 0
        while r < self.LPAD:
            g = min(tgroup, self.LPAD - r)
            g = (g // 16) * 16
            self.LGROUPS.append((r, g))
            r += g
        assert r == self.LPAD


def build_kernel(nc: bass.Bass, cfg: Cfg):
    W, H, C, KW = cfg.W, cfg.H, cfg.C, cfg.KW
    KMAX, SUMKW = cfg.KMAX, cfg.SUMKW
    TG, NG = cfg.TGROUP, cfg.NG
    LASTR = cfg.LASTR
    N = cfg.N

    # ---- I/O ----
    h_b = nc.declare_dram_parameter("h_b", [cfg.HPAD, P], BF16, isOutput=False)
    hloc_b = nc.declare_dram_parameter("hloc_b", [cfg.LPAD, P], BF16,
                                       isOutput=False)
    hres = nc.declare_dram_parameter("hres", [cfg.LPAD, P], F32,
                                     isOutput=False)
    idxp = nc.declare_dram_parameter("idxp", [P, 8 * SUMKW], I16,
                                      isOutput=False)
    maskp = nc.declare_dram_parameter("maskp", [P, SUMKW], BF16,
                                      isOutput=False)
    parp = nc.declare_dram_parameter("parp", [P, SUMKW], BF16,
                                     isOutput=False)
    constsA = nc.declare_dram_parameter("constsA", [P, 3], F32, isOutput=False)
    constsB = nc.declare_dram_parameter("constsB", [P, 2 * P], BF16,
                                        isOutput=False)
    constsC = nc.declare_dram_parameter("constsC", [64, P], BF16,
                                        isOutput=False)
    out = nc.declare_dram_parameter("out", [cfg.NPC, P], F32, isOutput=True)

    # ---- internal DRAM ----
    tab = nc.dram_tensor("tab", [cfg.TAB, P], BF16)
    st_in = nc.dram_tensor("st_in", [P, 2], F32)
    st_out = nc.dram_tensor("st_out", [P, 2], F32, addr_space="Shared")

    # pair view: row t = [xl-table-row 2t | row 2t+1], 512B per row
    tab_pair = tab[:].rearrange("(t b) f -> t (b f)", b=2)

    with tile.TileContext(nc) as tc:
        import contextlib
        with contextlib.ExitStack() as ctx:
            cst = ctx.enter_context(tc.tile_pool(name="cst", bufs=1))
            sb = ctx.enter_context(tc.tile_pool(name="sb", bufs=2))
            pG = ctx.enter_context(tc.tile_pool(name="pG", bufs=3))
            pZ = ctx.enter_context(tc.tile_pool(name="pZ", bufs=3))
            ps = ctx.enter_context(tc.tile_pool(name="ps", bufs=2,
                                                space="PSUM"))
            ps1 = ctx.enter_context(tc.tile_pool(name="ps1", bufs=1,
                                                 space="PSUM"))
            ps2 = ctx.enter_context(tc.tile_pool(name="ps2", bufs=1,
                                                 space="PSUM"))

            # ================= constants =================
            csA = cst.tile([P, 3], F32, tag="csA")
            nc.sync.dma_start(out=csA[:], in_=constsA[:])
            att_col = csA[:, 0:1]
            gam_col = csA[:, 1:2]
            bet_col = csA[:, 2:3]
            csB = cst.tile([P, 2 * P], BF16, tag="csB")
            nc.sync.dma_start(out=csB[:], in_=constsB[:])
            Wl_bf = csB[:, 0:P]
            Wr_bf = csB[:, P:2 * P]
            csC = cst.tile([64, P], BF16, tag="csC")
            nc.sync.dma_start(out=csC[:], in_=constsC[:])
            bl_row = csC[0:1, :]
            br_row = csC[32:33, :]

            ones1_bf = cst.tile([64, P], BF16, tag="ones1")
            nc.gpsimd.memset(ones1_bf[:], 1.0)
            ones_col = cst.tile([P, 1], F32, tag="ones_c")
            nc.gpsimd.memset(ones_col[:], 1.0)
            zero_row = cst.tile([1, P], BF16, tag="zrow")
            nc.gpsimd.memset(zero_row[:], 0.0)
            ident = cst.tile([P, P], F32, tag="ident")
            make_identity(nc, ident[:])
            eps_col = cst.tile([P, 1], F32, tag="epsc")
            nc.gpsimd.memset(eps_col[:], BN_EPS)
            ones_msk = cst.tile([P, 1], F32, tag="ones_m")
            if LASTR < P:
                pidx = cst.tile([P, 1], I32, tag="pidx")
                nc.gpsimd.iota(pidx[:], pattern=[[0, 1]], channel_multiplier=1)
                nc.vector.tensor_scalar(out=ones_msk[:], in0=pidx[:],
                                        scalar1=LASTR, scalar2=None,
                                        op0=mybir.AluOpType.is_lt)
            else:
                nc.gpsimd.memset(ones_msk[:], 1.0)

            # att replicated to all partitions (bf16)
            att_ps = ps.tile([P, P], F32, tag="ptr")
            nc.tensor.transpose(att_ps[:], att_col.to_broadcast([P, P]),
                                ident[:])
            att_rep = cst.tile([P, P], BF16, tag="attrep")
            nc.scalar.copy(att_rep[:], att_ps[:])

            # pinned phase-2 operands
            idx_pin = cst.tile([P, 8 * SUMKW], I16, tag="idx")
            nc.sync.dma_start(out=idx_pin[:], in_=idxp[:])
            msk_pin = cst.tile([P, SUMKW], BF16, tag="msk")
            nc.sync.dma_start(out=msk_pin[:], in_=maskp[:])
            par_pin = cst.tile([P, SUMKW], BF16, tag="par")
            nc.sync.dma_start(out=par_pin[:], in_=parp[:])
            xr_pin = cst.tile([P, cfg.LPAD], BF16, tag="xrp")
            outpre = cst.tile([P, cfg.LPAD], F32, tag="opre")

            # zero the lo-view sentinel row (tab[0]); hi sentinel (tab[SENT])
            # is written by phase 1 from zero-padded h rows, but write it
            # explicitly in case of nonzero bias_l.
            nc.sync.dma_start(out=tab[0:1, :], in_=zero_row[:])
            nc.sync.dma_start(out=tab[cfg.SENT:cfg.SENT + 1, :],
                              in_=zero_row[:])
            nc.sync.dma_start(out=tab[cfg.TAB - 1:cfg.TAB, :],
                              in_=zero_row[:])

            # ================= phase 1: xl table =================
            NT = TG // P  # tiles per group
            for g in range(NG):
                hT = sb.tile([P, TG], BF16, tag="hT")
                nc.sync.dma_start(out=hT[:], in_=h_b[g * TG:(g + 1) * TG, :],
                                  transpose=True)
                xt = sb.tile([P, TG], BF16, tag="xt")
                for jj in range(NT // 4):
                    pst = ps.tile([P, 4 * P], F32, tag="p1")
                    for j in range(4):
                        t = jj * 4 + j
                        nc.tensor.matmul(pst[:, j * P:(j + 1) * P],
                                         lhsT=hT[:, t * P:(t + 1) * P],
                                         rhs=Wl_bf, start=True,
                                         stop=not cfg.HAS_BL)
                        if cfg.HAS_BL:
                            nc.tensor.matmul(pst[:, j * P:(j + 1) * P],
                                             lhsT=ones1_bf[0:1, :], rhs=bl_row,
                                             start=False, stop=True)
                    nc.scalar.copy(xt[:, jj * 4 * P:(jj + 1) * 4 * P],
                                   pst[:])
                # store rows [g*TG+1, (g+1)*TG+1) — xl row j at tab[j+1]
                nc.scalar.dma_start(
                    out=tab[g * TG + 1:(g + 1) * TG + 1, :].rearrange(
                        "(a p) f -> p a f", p=P),
                    in_=xt[:].rearrange("p (a f) -> p a f", f=P))

            # ================= phase 1b: xr (local, stays in SBUF) ========
            for (r0, rows) in cfg.LGROUPS:
                hTl = sb.tile([P, TG], BF16, tag="hTl")
                nc.sync.dma_start(out=hTl[:, :rows],
                                  in_=hloc_b[r0:r0 + rows, :], transpose=True)
                nt = rows // P
                for jj in range(math.ceil(nt / 4)):
                    cols = min(4, nt - jj * 4)
                    pst = ps.tile([P, 4 * P], F32, tag="p1")
                    for j in range(cols):
                        t = jj * 4 + j
                        nc.tensor.matmul(pst[:, j * P:(j + 1) * P],
                                         lhsT=hTl[:, t * P:(t + 1) * P],
                                         rhs=Wr_bf, start=True,
                                         stop=not cfg.HAS_BR)
                        if cfg.HAS_BR:
                            nc.tensor.matmul(pst[:, j * P:(j + 1) * P],
                                             lhsT=ones1_bf[32:33, :],
                                             rhs=br_row,
                                             start=False, stop=True)
                    c0 = r0 + jj * 4 * P
                    nc.scalar.copy(
                        xr_pin[:, c0:c0 + cols * P], pst[:, :cols * P])

            tc.strict_bb_all_engine_barrier()

            # ================= phase 2: edges =================
            stats_ps = ps1.tile([P, 1], F32, tag="stats")
            stats2_ps = ps2.tile([P, 1], F32, tag="stats2")
            koff = 0
            for w in range(W):
                Kw = KW[w]
                NI = P * Kw
                g2 = pG.tile([P, KMAX, 2 * P], BF16, tag="G2")
                nc.gpsimd.dma_gather(
                    g2[:, :Kw, :], tab_pair,
                    idx_pin[:, 8 * koff:8 * (koff + Kw)],
                    num_idxs=NI, num_idxs_reg=NI, elem_size=2 * P,
                    elem_step=2 * P, single_packet=False)
                # y = parity-select(g2) + xr[dst-row]
                y = pG.tile([P, KMAX, P], BF16, tag="y")
                nc.vector.tensor_tensor(out=y[:, :Kw, :],
                                        in0=g2[:, :Kw, P:2 * P],
                                        in1=g2[:, :Kw, 0:P],
                                        op=mybir.AluOpType.subtract)
                nc.vector.tensor_tensor(
                    out=y[:, :Kw, :], in0=y[:, :Kw, :],
                    in1=par_pin[:, koff:koff + Kw][:, :, None]
                        .to_broadcast([P, Kw, P]),
                    op=mybir.AluOpType.mult)
                nc.vector.tensor_tensor(out=y[:, :Kw, :], in0=y[:, :Kw, :],
                                        in1=g2[:, :Kw, 0:P],
                                        op=mybir.AluOpType.add)
                nc.vector.tensor_tensor(
                    out=y[:, :Kw, :], in0=y[:, :Kw, :],
                    in1=xr_pin[:, None, w * P:(w + 1) * P]
                        .to_broadcast([P, Kw, P]),
                    op=mybir.AluOpType.add)
                # z = LeakyReLU(y)
                z = pZ.tile([P, KMAX, P], BF16, tag="z")
                nc.scalar.activation(z[:, :Kw, :], y[:, :Kw, :],
                                     mybir.ActivationFunctionType.Prelu,
                                     alpha=NEG_SLOPE)
                # zz = z * att ; s = sum_c zz
                w2 = pZ.tile([P, KMAX, P], BF16, tag="w2")
                nc.vector.tensor_tensor(
                    out=w2[:, :Kw, :], in0=z[:, :Kw, :],
                    in1=att_rep[:, None, :].to_broadcast([P, Kw, P]),
                    op=mybir.AluOpType.mult)
                s = pZ.tile([P, KMAX, H], F32, tag="s")
                nc.vector.tensor_reduce(
                    out=s[:, :Kw, :][:, :, :, None],
                    in_=w2[:, :Kw, :].rearrange("p k (h c) -> p k h c", c=C),
                    op=mybir.AluOpType.add, axis=mybir.AxisListType.X)
                # alpha = exp(s) * mask
                al = pZ.tile([P, KMAX, H], BF16, tag="al")
                nc.scalar.activation(al[:, :Kw, :], s[:, :Kw, :],
                                     mybir.ActivationFunctionType.Exp)
                nc.vector.tensor_tensor(
                    out=al[:, :Kw, :], in0=al[:, :Kw, :],
                    in1=msk_pin[:, koff:koff + Kw][:, :, None]
                        .to_broadcast([P, Kw, H]),
                    op=mybir.AluOpType.mult)
                # ay = y * alpha ; acc = sum_k ay ; den = sum_k alpha
                nc.vector.tensor_tensor(
                    out=w2[:, :Kw, :].rearrange("p k (h c) -> p k h c", c=C),
                    in0=y[:, :Kw, :].rearrange("p k (h c) -> p k h c", c=C),
                    in1=al[:, :Kw, :][:, :, :, None].to_broadcast([P, Kw, H, C]),
                    op=mybir.AluOpType.mult)
                acc = pZ.tile([P, P], F32, tag="acc")
                nc.vector.tensor_reduce(
                    out=acc[:][:, :, None],
                    in_=w2[:, :Kw, :].rearrange("p k f -> p f k"),
                    op=mybir.AluOpType.add, axis=mybir.AxisListType.X)
                den = pZ.tile([P, H], F32, tag="den")
                nc.vector.tensor_reduce(
                    out=den[:][:, :, None],
                    in_=al[:, :Kw, :].rearrange("p k h -> p h k"),
                    op=mybir.AluOpType.add, axis=mybir.AxisListType.X)
                # out_pre = acc / den - xr   (den > 0 by construction:
                # every row, incl. padding rows, has at least one unmasked
                # slot: self-loop for real nodes, slot 0 for pad rows)
                rec = pZ.tile([P, H], F32, tag="rec")
                nc.vector.reciprocal(rec[:], den[:])
                op_sl = outpre[:, w * P:(w + 1) * P]
                nc.vector.tensor_tensor(
                    out=op_sl.rearrange("p (h c) -> p h c", c=C),
                    in0=acc[:].rearrange("p (h c) -> p h c", c=C),
                    in1=rec[:, :, None].to_broadcast([P, H, C]),
                    op=mybir.AluOpType.mult)
                nc.vector.tensor_tensor(
                    out=op_sl, in0=op_sl, in1=xr_pin[:, w * P:(w + 1) * P],
                    op=mybir.AluOpType.subtract)
                # BN stats
                sq = pZ.tile([P, P], F32, tag="sq")
                nc.scalar.square(sq[:], op_sl)
                stat_ones = ones_msk if w == W - 1 else ones_col
                nc.tensor.matmul(stats_ps[:], lhsT=op_sl,
                                 rhs=stat_ones[:],
                                 start=(w == 0), stop=(w == W - 1))
                nc.tensor.matmul(stats2_ps[:], lhsT=sq[:],
                                 rhs=stat_ones[:],
                                 start=(w == 0), stop=(w == W - 1))
                koff += Kw

            # ================= phase 3: BN stats AllReduce =================
            st_sb = sb.tile([P, 2], F32, tag="stsb")
            nc.scalar.copy(st_sb[:, 0:1], stats_ps[:])
            nc.scalar.copy(st_sb[:, 1:2], stats2_ps[:])
            nc.sync.dma_start(out=st_in[:], in_=st_sb[:])
            tc.strict_bb_all_engine_barrier()
            nc.gpsimd.collective_compute(
                "AllReduce", mybir.AluOpType.add,
                replica_groups=[list(range(cfg.n_cores))],
                ins=[st_in[:]], outs=[st_out[:]])
            tc.strict_bb_all_engine_barrier()
            st_all = sb.tile([P, 2], F32, tag="stall")
            nc.sync.dma_start(out=st_all[:], in_=st_out[:])

            # A = gamma * rsqrt(var+eps); B = beta - A*mu  (y = A*x + B)
            mu_c = sb.tile([P, 1], F32, tag="mu")
            nc.scalar.mul(mu_c[:], st_all[:, 0:1], 1.0 / N)
            ex2 = sb.tile([P, 1], F32, tag="ex2")
            nc.scalar.mul(ex2[:], st_all[:, 1:2], 1.0 / N)
            mu2 = sb.tile([P, 1], F32, tag="mu2")
            nc.scalar.square(mu2[:], mu_c[:])
            var_c = sb.tile([P, 1], F32, tag="var")
            nc.vector.tensor_tensor(out=var_c[:], in0=ex2[:], in1=mu2[:],
                                    op=mybir.AluOpType.subtract)
            sd = sb.tile([P, 1], F32, tag="sd")
            nc.scalar.activation(sd[:], var_c[:],
                                 mybir.ActivationFunctionType.Sqrt,
                                 bias=eps_col[:])
            rsd = sb.tile([P, 1], F32, tag="rsd")
            nc.vector.reciprocal(rsd[:], sd[:])
            A_c = sb.tile([P, 1], F32, tag="Ac")
            nc.vector.tensor_tensor(out=A_c[:], in0=gam_col, in1=rsd[:],
                                    op=mybir.AluOpType.mult)
            Amu = sb.tile([P, 1], F32, tag="Amu")
            nc.vector.tensor_tensor(out=Amu[:], in0=A_c[:], in1=mu_c[:],
                                    op=mybir.AluOpType.mult)
            B_c = sb.tile([P, 1], F32, tag="Bc")
            nc.vector.tensor_tensor(out=B_c[:], in0=bet_col, in1=Amu[:],
                                    op=mybir.AluOpType.subtract)

            A_ps = ps.tile([P, P], F32, tag="ptr")
            nc.tensor.transpose(A_ps[:], A_c[:].to_broadcast([P, P]), ident[:])
            A_rep = cst.tile([P, P], F32, tag="Arep")
            nc.scalar.copy(A_rep[:], A_ps[:])
            B_ps = ps.tile([P, P], F32, tag="ptr")
            nc.tensor.transpose(B_ps[:], B_c[:].to_broadcast([P, P]), ident[:])
            B_rep = cst.tile([P, P], F32, tag="Brep")
            nc.scalar.copy(B_rep[:], B_ps[:])

            # ================= phase 4: BN apply + relu + residual ==========
            for w in range(W):
                rows = P if w < W - 1 else LASTR
                t1 = sb.tile([P, P], F32, tag="t1")
                nc.vector.tensor_tensor(out=t1[:], in0=outpre[:, w * P:(w + 1) * P],
                                        in1=A_rep[:], op=mybir.AluOpType.mult)
                t2 = sb.tile([P, P], F32, tag="t2")
                nc.vector.tensor_tensor(out=t2[:], in0=t1[:], in1=B_rep[:],
                                        op=mybir.AluOpType.add)
                r = sb.tile([P, P], F32, tag="r")
                nc.scalar.activation(r[:], t2[:],
                                     mybir.ActivationFunctionType.Relu)
                hr = sb.tile([P, P], F32, tag="hr")
                nc.sync.dma_start(out=hr[:],
                                  in_=hres[w * P:(w + 1) * P, :])
                o = sb.tile([P, P], F32, tag="o")
                nc.vector.tensor_tensor(out=o[:], in0=r[:], in1=hr[:],
                                        op=mybir.AluOpType.add)
                nc.sync.dma_start(out=out[w * P:w * P + rows, :],
                                  in_=o[:rows, :])
    return nc


def host_prepare(h, edge_index, W_l, W_r, bias_l, bias_r, att,
                 bias_out, gamma, beta, n_cores=8):
    N, D = h.shape
    H, C = att.shape
    E = edge_index.shape[1]
    h = np.asarray(h, np.float32)
    ei = np.asarray(edge_index)

    loops = np.arange(N, dtype=np.int64)
    src = np.concatenate([ei[0], loops]).astype(np.int64)
    dst = np.concatenate([ei[1], loops]).astype(np.int64)
    order = np.argsort(dst, kind="stable")
    src_s = src[order].astype(np.int32)
    dst_s = dst[order].astype(np.int32)

    NPC = N // n_cores
    W = math.ceil(NPC / P)
    LPAD = W * P
    bounds = np.searchsorted(dst_s, np.arange(0, N + 1, NPC))

    # per-core degree sort
    percore = []
    kw_all = np.zeros((n_cores, W), np.int64)
    for k in range(n_cores):
        lo, hi = bounds[k], bounds[k + 1]
        s_k = src_s[lo:hi]
        d_k = dst_s[lo:hi] - k * NPC
        deg = np.bincount(d_k, minlength=NPC)
        perm = np.argsort(-deg, kind="stable")   # node order, high-deg first
        dsort = deg[perm]
        dpad = np.concatenate([dsort, np.zeros(LPAD - NPC, np.int64)])
        kw_all[k] = dpad.reshape(W, P).max(axis=1)
        starts = np.concatenate([[0], np.cumsum(deg)])
        percore.append((s_k, deg, perm, starts))
    KW = kw_all.max(axis=0)
    KW = np.maximum(KW, 1)

    cfg = Cfg(N=N, D=D, H=H, E=E, n_cores=n_cores, KW=KW,
              has_bl=bool(np.any(np.asarray(bias_l))),
              has_br=bool(np.any(np.asarray(bias_r))))
    SUMKW = cfg.SUMKW

    # shared tensors
    h_b = np.zeros((cfg.HPAD, P), NPBF16)
    h_b[:N] = h.astype(NPBF16)
    constsA = np.zeros((P, 3), np.float32)
    constsA[:, 0] = np.asarray(att, np.float32).reshape(-1)
    constsA[:, 1] = np.asarray(gamma, np.float32)
    constsA[:, 2] = np.asarray(beta, np.float32)
    constsB = np.zeros((P, 2 * P), np.float32)
    constsB[:, 0:P] = np.asarray(W_l, np.float32)
    constsB[:, P:2 * P] = np.asarray(W_r, np.float32)
    constsB = constsB.astype(NPBF16)
    constsC = np.zeros((64, P), np.float32)
    constsC[0] = np.asarray(bias_l, np.float32)
    constsC[32] = np.asarray(bias_r, np.float32)
    constsC = constsC.astype(NPBF16)

    in_maps = []
    perms = []
    for k in range(n_cores):
        s_k, deg, perm, starts = percore[k]
        perms.append(perm)
        # slot tables [LPAD rows, KW[w] cols per window]
        idx = np.zeros((P, 8 * SUMKW), np.int16)
        msk = np.zeros((P, SUMKW), np.float32)
        par = np.zeros((P, SUMKW), np.float32)
        koff = 0
        for w in range(W):
            Kw = int(KW[w])
            NI = P * Kw
            # srcs[p, j] = j-th edge's src of node perm[w*128+p]
            i16 = np.zeros(NI, np.int32)
            for p in range(P):
                v = w * P + p
                if v >= NPC:
                    # pad row: slot 0 stays idx 0 / par 0 (-> zero half),
                    # unmask it so den >= 1 and no divide-by-zero
                    msk[p, koff] = 1.0
                    continue
                node = perm[v]
                e0, e1 = starts[node], starts[node + 1]
                d = e1 - e0
                if d == 0:
                    msk[p, koff] = 1.0
                    continue
                srcs = s_k[e0:e1].astype(np.int64)
                pos = np.arange(d) * P + p      # slot i = k*128+p
                t = srcs + 1                    # xl[src] at table row src+1
                i16[pos] = t >> 1
                par[p, koff:koff + d] = (t & 1).astype(np.float32)
                msk[p, koff:koff + d] = 1.0
            # wrap: [16, NI/16] then replicate to 128 partitions
            blk = i16.reshape(-1, 16).T.astype(np.int16)
            idx[:, 8 * koff:8 * (koff + Kw)] = np.tile(blk, (8, 1))
            koff += Kw

        gidx = k * NPC + perm                   # global ids, perm order
        hloc_b = np.zeros((LPAD, P), NPBF16)
        hloc_b[:NPC] = h[gidx].astype(NPBF16)
        hres = np.zeros((LPAD, P), np.float32)
        hres[:NPC] = h[gidx]

        in_maps.append({
            "h_b": h_b, "hloc_b": hloc_b, "hres": hres,
            "idxp": idx,
            "maskp": msk.astype(NPBF16),
            "parp": par.astype(NPBF16),
            "constsA": constsA, "constsB": constsB, "constsC": constsC,
        })
    return cfg, in_maps, perms


def kernel(h, edge_index, W_l, W_r, bias_l, bias_r, att,
           bias_out, gamma, beta):
    n_cores = 8
    cfg, in_maps, perms = host_prepare(
        h, edge_index, W_l, W_r, bias_l, bias_r, att, bias_out, gamma, beta,
        n_cores=n_cores)
    nc = bacc.Bacc()
    build_kernel(nc, cfg)
    nc.compile()
    res = run_bass_kernel_spmd(nc, in_maps, core_ids=list(range(n_cores)))
    N = cfg.N
    out_full = np.empty((N, cfg.D), np.float32)
    for k in range(n_cores):
        gidx = k * cfg.NPC + perms[k]
        out_full[gidx] = res.results[k]["out"]
    return out_full


# revision 9
# speedup vs baseline: 1.2260x; 1.0446x over previous
"""GATv2 layer (N=50000, D=128, H=4, E=600000) on 8 trn2 NeuronCores — v2.

Architecture (vs v1 baseline):
- Degree-aligned edge layout: dst nodes are sorted by in-degree per core and
  assigned to 128-row windows; partition row p of window w holds ALL edges of
  one dst node, padded to the window max degree Kw.  This makes the segment
  softmax and the weighted aggregation per-partition reductions — no
  selection-matrix matmuls, no per-edge xr gather.
- bf16 xl table + dma_gather (InstDMAGatherAnt, int16 idx) instead of
  row-at-a-time indirect DMA: one gather op covers a whole window (128*Kw
  rows).  int16 range is handled with a lo/hi split view of one table plus
  dedicated zero rows (exactly one of the two gathers returns nonzero and
  they are summed on DVE).
- Phase 1 (xl = h@W_l) uses host-cast bf16 h loaded via HWDGE DMA-transpose,
  bf16 matmuls, bf16 table stores.
- Softmax without max-subtraction (scores are O(+-10)); bias_out dropped
  (absorbed exactly by BatchNorm mean subtraction); normalization folded to
  a per-node divide after aggregation.  BN stats AllReduced across cores.
"""

import math
import numpy as np
import ml_dtypes

import concourse.bass as bass
import concourse.bacc as bacc
import concourse.mybir as mybir
import concourse.tile as tile
from concourse.masks import make_identity
from concourse.bass_utils import run_bass_kernel_spmd

P = 128
F32 = mybir.dt.float32
BF16 = mybir.dt.bfloat16
I16 = mybir.dt.int16
I32 = mybir.dt.int32

NEG_SLOPE = 0.2
BN_EPS = 1e-5
NPBF16 = ml_dtypes.bfloat16


class Cfg:
    def __init__(self, N, D, H, E, n_cores, KW, locap=32767, hibase=None,
                 tgroup=2048, has_bl=False, has_br=False):
        assert D == 128
        self.N, self.D, self.H, self.E = N, D, H, E
        self.C = D // H
        self.n_cores = n_cores
        assert N % n_cores == 0
        self.NPC = N // n_cores
        self.W = math.ceil(self.NPC / P)
        self.LASTR = self.NPC - P * (self.W - 1)
        self.KW = list(KW)                       # per-window max degree
        assert len(self.KW) == self.W
        self.KMAX = max(self.KW)
        self.SUMKW = sum(self.KW)
        self.TGROUP = tgroup                     # rows per phase-1 group
        self.HPAD = math.ceil((N + 2) / tgroup) * tgroup
        self.NG = self.HPAD // tgroup
        self.TAB = self.HPAD + 2                 # xl table rows (xl at +1)
        self.LOCAP = locap                       # lo view: src <= locap-1
        self.SENT = N + 1                        # hi zero row (table pos)
        if hibase is None:
            hibase = max(0, self.SENT - locap)
        self.HIBASE = hibase
        assert self.SENT - hibase <= locap
        assert locap + 1 <= self.TAB
        self.HVIEW = min(locap + 1, self.TAB - hibase)  # hi view rows
        assert hibase + self.HVIEW >= self.SENT + 1
        self.TAB2 = self.TAB // 2                # pair rows (512B each)
        assert (self.N + 1) // 2 + 1 < 32767
        self.HAS_BL = has_bl
        self.HAS_BR = has_br
        # local node rows padded to full windows
        self.LPAD = self.W * P
        self.LGROUPS = []                        # phase-1b transpose groups
        r = 0
        while r < self.LPAD:
            g = min(tgroup, self.LPAD - r)
            g = (g // 16) * 16
            self.LGROUPS.append((r, g))
            r += g
        assert r == self.LPAD


def build_kernel(nc: bass.Bass, cfg: Cfg):
    W, H, C, KW = cfg.W, cfg.H, cfg.C, cfg.KW
    KMAX, SUMKW = cfg.KMAX, cfg.SUMKW
    TG, NG = cfg.TGROUP, cfg.NG
    LASTR = cfg.LASTR
    N = cfg.N

    # ---- I/O ----
    h_b = nc.declare_dram_parameter("h_b", [cfg.HPAD, P], BF16, isOutput=False)
    hloc_b = nc.declare_dram_parameter("hloc_b", [cfg.LPAD, P], BF16,
                                       isOutput=False)
    hres = nc.declare_dram_parameter("hres", [cfg.LPAD, P], F32,
                                     isOutput=False)
    idxp = nc.declare_dram_parameter("idxp", [P, 8 * SUMKW], I16,
                                      isOutput=False)
    maskp = nc.declare_dram_parameter("maskp", [P, SUMKW], BF16,
                                      isOutput=False)
    parp = nc.declare_dram_parameter("parp", [P, SUMKW], BF16,
                                     isOutput=False)
    constsA = nc.declare_dram_parameter("constsA", [P, 3], F32, isOutput=False)
    constsB = nc.declare_dram_parameter("constsB", [P, 2 * P], BF16,
                                        isOutput=False)
    constsC = nc.declare_dram_parameter("constsC", [64, P], BF16,
                                        isOutput=False)
    out = nc.declare_dram_parameter("out", [cfg.NPC, P], F32, isOutput=True)

    # ---- internal DRAM ----
    tab = nc.dram_tensor("tab", [cfg.TAB, P], BF16)
    st_in = nc.dram_tensor("st_in", [P, 2], F32)
    st_out = nc.dram_tensor("st_out", [P, 2], F32, addr_space="Shared")

    # pair view: row t = [xl-table-row 2t | row 2t+1], 512B per row
    tab_pair = tab[:].rearrange("(t b) f -> t (b f)", b=2)

    with tile.TileContext(nc) as tc:
        import contextlib
        with contextlib.ExitStack() as ctx:
            cst = ctx.enter_context(tc.tile_pool(name="cst", bufs=1))
            sb = ctx.enter_context(tc.tile_pool(name="sb", bufs=2))
            pG = ctx.enter_context(tc.tile_pool(name="pG", bufs=3))
            pZ = ctx.enter_context(tc.tile_pool(name="pZ", bufs=3))
            ps = ctx.enter_context(tc.tile_pool(name="ps", bufs=2,
                                                space="PSUM"))
            ps1 = ctx.enter_context(tc.tile_pool(name="ps1", bufs=1,
                                                 space="PSUM"))
            ps2 = ctx.enter_context(tc.tile_pool(name="ps2", bufs=1,
                                                 space="PSUM"))

            # ================= constants =================
            csA = cst.tile([P, 3], F32, tag="csA")
            nc.sync.dma_start(out=csA[:], in_=constsA[:])
            att_col = csA[:, 0:1]
            gam_col = csA[:, 1:2]
            bet_col = csA[:, 2:3]
            csB = cst.tile([P, 2 * P], BF16, tag="csB")
            nc.sync.dma_start(out=csB[:], in_=constsB[:])
            Wl_bf = csB[:, 0:P]
            Wr_bf = csB[:, P:2 * P]
            csC = cst.tile([64, P], BF16, tag="csC")
            nc.sync.dma_start(out=csC[:], in_=constsC[:])
            bl_row = csC[0:1, :]
            br_row = csC[32:33, :]

            ones1_bf = cst.tile([64, P], BF16, tag="ones1")
            nc.gpsimd.memset(ones1_bf[:], 1.0)
            ones_col = cst.tile([P, 1], BF16, tag="ones_c")
            nc.gpsimd.memset(ones_col[:], 1.0)
            zero_row = cst.tile([1, P], BF16, tag="zrow")
            nc.gpsimd.memset(zero_row[:], 0.0)
            ident = cst.tile([P, P], F32, tag="ident")
            make_identity(nc, ident[:])
            eps_col = cst.tile([P, 1], F32, tag="epsc")
            nc.gpsimd.memset(eps_col[:], BN_EPS)
            ones_msk = cst.tile([P, 1], BF16, tag="ones_m")
            if LASTR < P:
                pidx = cst.tile([P, 1], I32, tag="pidx")
                nc.gpsimd.iota(pidx[:], pattern=[[0, 1]], channel_multiplier=1)
                nc.vector.tensor_scalar(out=ones_msk[:], in0=pidx[:],
                                        scalar1=LASTR, scalar2=None,
                                        op0=mybir.AluOpType.is_lt)
            else:
                nc.gpsimd.memset(ones_msk[:], 1.0)

            # att replicated to all partitions (bf16)
            att_ps = ps.tile([P, P], F32, tag="ptr")
            nc.tensor.transpose(att_ps[:], att_col.to_broadcast([P, P]),
                                ident[:])
            att_rep = cst.tile([P, P], BF16, tag="attrep")
            nc.scalar.copy(att_rep[:], att_ps[:])

            # pinned phase-2 operands
            idx_pin = cst.tile([P, 8 * SUMKW], I16, tag="idx")
            nc.sync.dma_start(out=idx_pin[:], in_=idxp[:])
            msk_pin = cst.tile([P, SUMKW], BF16, tag="msk")
            nc.sync.dma_start(out=msk_pin[:], in_=maskp[:])
            par_pin = cst.tile([P, SUMKW], BF16, tag="par")
            nc.sync.dma_start(out=par_pin[:], in_=parp[:])
            xr_pin = cst.tile([P, cfg.LPAD], BF16, tag="xrp")
            outpre = cst.tile([P, cfg.LPAD], BF16, tag="opre")

            # zero the lo-view sentinel row (tab[0]); hi sentinel (tab[SENT])
            # is written by phase 1 from zero-padded h rows, but write it
            # explicitly in case of nonzero bias_l.
            nc.sync.dma_start(out=tab[0:1, :], in_=zero_row[:])
            nc.sync.dma_start(out=tab[cfg.SENT:cfg.SENT + 1, :],
                              in_=zero_row[:])
            nc.sync.dma_start(out=tab[cfg.TAB - 1:cfg.TAB, :],
                              in_=zero_row[:])

            # ================= phase 1: xl table =================
            NT = TG // P  # tiles per group
            for g in range(NG):
                hT = sb.tile([P, TG], BF16, tag="hT")
                nc.sync.dma_start(out=hT[:], in_=h_b[g * TG:(g + 1) * TG, :],
                                  transpose=True)
                xt = sb.tile([P, TG], BF16, tag="xt")
                for jj in range(NT // 4):
                    pst = ps.tile([P, 4 * P], F32, tag="p1")
                    for j in range(4):
                        t = jj * 4 + j
                        nc.tensor.matmul(pst[:, j * P:(j + 1) * P],
                                         lhsT=hT[:, t * P:(t + 1) * P],
                                         rhs=Wl_bf, start=True,
                                         stop=not cfg.HAS_BL)
                        if cfg.HAS_BL:
                            nc.tensor.matmul(pst[:, j * P:(j + 1) * P],
                                             lhsT=ones1_bf[0:1, :], rhs=bl_row,
                                             start=False, stop=True)
                    nc.scalar.copy(xt[:, jj * 4 * P:(jj + 1) * 4 * P],
                                   pst[:])
                # store rows [g*TG+1, (g+1)*TG+1) — xl row j at tab[j+1]
                nc.scalar.dma_start(
                    out=tab[g * TG + 1:(g + 1) * TG + 1, :].rearrange(
                        "(a p) f -> p a f", p=P),
                    in_=xt[:].rearrange("p (a f) -> p a f", f=P))

            # ================= phase 1b: xr (local, stays in SBUF) ========
            for (r0, rows) in cfg.LGROUPS:
                hTl = sb.tile([P, TG], BF16, tag="hTl")
                nc.sync.dma_start(out=hTl[:, :rows],
                                  in_=hloc_b[r0:r0 + rows, :], transpose=True)
                nt = rows // P
                for jj in range(math.ceil(nt / 4)):
                    cols = min(4, nt - jj * 4)
                    pst = ps.tile([P, 4 * P], F32, tag="p1")
                    for j in range(cols):
                        t = jj * 4 + j
                        nc.tensor.matmul(pst[:, j * P:(j + 1) * P],
                                         lhsT=hTl[:, t * P:(t + 1) * P],
                                         rhs=Wr_bf, start=True,
                                         stop=not cfg.HAS_BR)
                        if cfg.HAS_BR:
                            nc.tensor.matmul(pst[:, j * P:(j + 1) * P],
                                             lhsT=ones1_bf[32:33, :],
                                             rhs=br_row,
                                             start=False, stop=True)
                    c0 = r0 + jj * 4 * P
                    nc.scalar.copy(
                        xr_pin[:, c0:c0 + cols * P], pst[:, :cols * P])

            tc.strict_bb_all_engine_barrier()

            # ================= phase 2: edges =================
            stats_ps = ps1.tile([P, 1], F32, tag="stats")
            stats2_ps = ps2.tile([P, 1], F32, tag="stats2")
            koff = 0
            for w in range(W):
                Kw = KW[w]
                NI = P * Kw
                g2 = pG.tile([P, KMAX, 2 * P], BF16, tag="G2")
                nc.gpsimd.dma_gather(
                    g2[:, :Kw, :], tab_pair,
                    idx_pin[:, 8 * koff:8 * (koff + Kw)],
                    num_idxs=NI, num_idxs_reg=NI, elem_size=2 * P,
                    elem_step=2 * P, single_packet=False)
                # y = parity-select(g2) + xr[dst-row]
                y = pG.tile([P, KMAX, P], BF16, tag="y")
                nc.vector.tensor_tensor(out=y[:, :Kw, :],
                                        in0=g2[:, :Kw, P:2 * P],
                                        in1=g2[:, :Kw, 0:P],
                                        op=mybir.AluOpType.subtract)
                nc.vector.tensor_tensor(
                    out=y[:, :Kw, :], in0=y[:, :Kw, :],
                    in1=par_pin[:, koff:koff + Kw][:, :, None]
                        .to_broadcast([P, Kw, P]),
                    op=mybir.AluOpType.mult)
                nc.vector.tensor_tensor(out=y[:, :Kw, :], in0=y[:, :Kw, :],
                                        in1=g2[:, :Kw, 0:P],
                                        op=mybir.AluOpType.add)
                nc.vector.tensor_tensor(
                    out=y[:, :Kw, :], in0=y[:, :Kw, :],
                    in1=xr_pin[:, None, w * P:(w + 1) * P]
                        .to_broadcast([P, Kw, P]),
                    op=mybir.AluOpType.add)
                # z = LeakyReLU(y)
                z = pZ.tile([P, KMAX, P], BF16, tag="z")
                nc.scalar.activation(z[:, :Kw, :], y[:, :Kw, :],
                                     mybir.ActivationFunctionType.Prelu,
                                     alpha=NEG_SLOPE)
                # zz = z * att ; s = sum_c zz
                w2 = pZ.tile([P, KMAX, P], BF16, tag="w2")
                nc.vector.tensor_tensor(
                    out=w2[:, :Kw, :], in0=z[:, :Kw, :],
                    in1=att_rep[:, None, :].to_broadcast([P, Kw, P]),
                    op=mybir.AluOpType.mult)
                s = pZ.tile([P, KMAX, H], F32, tag="s")
                nc.vector.tensor_reduce(
                    out=s[:, :Kw, :][:, :, :, None],
                    in_=w2[:, :Kw, :].rearrange("p k (h c) -> p k h c", c=C),
                    op=mybir.AluOpType.add, axis=mybir.AxisListType.X)
                # alpha = exp(s) * mask
                al = pZ.tile([P, KMAX, H], BF16, tag="al")
                nc.scalar.activation(al[:, :Kw, :], s[:, :Kw, :],
                                     mybir.ActivationFunctionType.Exp)
                nc.vector.tensor_tensor(
                    out=al[:, :Kw, :], in0=al[:, :Kw, :],
                    in1=msk_pin[:, koff:koff + Kw][:, :, None]
                        .to_broadcast([P, Kw, H]),
                    op=mybir.AluOpType.mult)
                # ay = y * alpha ; acc = sum_k ay ; den = sum_k alpha
                nc.vector.tensor_tensor(
                    out=w2[:, :Kw, :].rearrange("p k (h c) -> p k h c", c=C),
                    in0=y[:, :Kw, :].rearrange("p k (h c) -> p k h c", c=C),
                    in1=al[:, :Kw, :][:, :, :, None].to_broadcast([P, Kw, H, C]),
                    op=mybir.AluOpType.mult)
                acc = pZ.tile([P, P], F32, tag="acc")
                nc.vector.tensor_reduce(
                    out=acc[:][:, :, None],
                    in_=w2[:, :Kw, :].rearrange("p k f -> p f k"),
                    op=mybir.AluOpType.add, axis=mybir.AxisListType.X)
                den = pZ.tile([P, H], F32, tag="den")
                nc.vector.tensor_reduce(
                    out=den[:][:, :, None],
                    in_=al[:, :Kw, :].rearrange("p k h -> p h k"),
                    op=mybir.AluOpType.add, axis=mybir.AxisListType.X)
                # out_pre = acc / den - xr   (den > 0 by construction:
                # every row, incl. padding rows, has at least one unmasked
                # slot: self-loop for real nodes, slot 0 for pad rows)
                rec = pZ.tile([P, H], F32, tag="rec")
                nc.vector.reciprocal(rec[:], den[:])
                op_sl = outpre[:, w * P:(w + 1) * P]
                nc.vector.tensor_tensor(
                    out=op_sl.rearrange("p (h c) -> p h c", c=C),
                    in0=acc[:].rearrange("p (h c) -> p h c", c=C),
                    in1=rec[:, :, None].to_broadcast([P, H, C]),
                    op=mybir.AluOpType.mult)
                nc.vector.tensor_tensor(
                    out=op_sl, in0=op_sl, in1=xr_pin[:, w * P:(w + 1) * P],
                    op=mybir.AluOpType.subtract)
                # BN stats
                sq = pZ.tile([P, P], BF16, tag="sq")
                nc.scalar.square(sq[:], op_sl)
                stat_ones = ones_msk if w == W - 1 else ones_col
                nc.tensor.matmul(stats_ps[:], lhsT=op_sl,
                                 rhs=stat_ones[:],
                                 start=(w == 0), stop=(w == W - 1))
                nc.tensor.matmul(stats2_ps[:], lhsT=sq[:],
                                 rhs=stat_ones[:],
                                 start=(w == 0), stop=(w == W - 1))
                koff += Kw

            # ================= phase 3: BN stats AllReduce =================
            st_sb = sb.tile([P, 2], F32, tag="stsb")
            nc.scalar.copy(st_sb[:, 0:1], stats_ps[:])
            nc.scalar.copy(st_sb[:, 1:2], stats2_ps[:])
            nc.sync.dma_start(out=st_in[:], in_=st_sb[:])
            tc.strict_bb_all_engine_barrier()
            nc.gpsimd.collective_compute(
                "AllReduce", mybir.AluOpType.add,
                replica_groups=[list(range(cfg.n_cores))],
                ins=[st_in[:]], outs=[st_out[:]])
            tc.strict_bb_all_engine_barrier()
            st_all = sb.tile([P, 2], F32, tag="stall")
            nc.sync.dma_start(out=st_all[:], in_=st_out[:])

            # A = gamma * rsqrt(var+eps); B = beta - A*mu  (y = A*x + B)
            mu_c = sb.tile([P, 1], F32, tag="mu")
            nc.scalar.mul(mu_c[:], st_all[:, 0:1], 1.0 / N)
            ex2 = sb.tile([P, 1], F32, tag="ex2")
            nc.scalar.mul(ex2[:], st_all[:, 1:2], 1.0 / N)
            mu2 = sb.tile([P, 1], F32, tag="mu2")
            nc.scalar.square(mu2[:], mu_c[:])
            var_c = sb.tile([P, 1], F32, tag="var")
            nc.vector.tensor_tensor(out=var_c[:], in0=ex2[:], in1=mu2[:],
                                    op=mybir.AluOpType.subtract)
            sd = sb.tile([P, 1], F32, tag="sd")
            nc.scalar.activation(sd[:], var_c[:],
                                 mybir.ActivationFunctionType.Sqrt,
                                 bias=eps_col[:])
            rsd = sb.tile([P, 1], F32, tag="rsd")
            nc.vector.reciprocal(rsd[:], sd[:])
            A_c = sb.tile([P, 1], F32, tag="Ac")
            nc.vector.tensor_tensor(out=A_c[:], in0=gam_col, in1=rsd[:],
                                    op=mybir.AluOpType.mult)
            Amu = sb.tile([P, 1], F32, tag="Amu")
            nc.vector.tensor_tensor(out=Amu[:], in0=A_c[:], in1=mu_c[:],
                                    op=mybir.AluOpType.mult)
            B_c = sb.tile([P, 1], F32, tag="Bc")
            nc.vector.tensor_tensor(out=B_c[:], in0=bet_col, in1=Amu[:],
                                    op=mybir.AluOpType.subtract)

            A_ps = ps.tile([P, P], F32, tag="ptr")
            nc.tensor.transpose(A_ps[:], A_c[:].to_broadcast([P, P]), ident[:])
            A_rep = cst.tile([P, P], F32, tag="Arep")
            nc.scalar.copy(A_rep[:], A_ps[:])
            B_ps = ps.tile([P, P], F32, tag="ptr")
            nc.tensor.transpose(B_ps[:], B_c[:].to_broadcast([P, P]), ident[:])
            B_rep = cst.tile([P, P], F32, tag="Brep")
            nc.scalar.copy(B_rep[:], B_ps[:])

            # ================= phase 4: BN apply + relu + residual ==========
            FB = 3
            wfull = W - 1
            wb = 0
            while wb < wfull:
                nb = min(FB, wfull - wb)
                cols = nb * P
                c0 = wb * P
                t1 = sb.tile([P, FB * P], F32, tag="t1")
                nc.vector.tensor_tensor(
                    out=t1[:, :cols].rearrange("p (a f) -> p a f", f=P),
                    in0=outpre[:, c0:c0 + cols].rearrange(
                        "p (a f) -> p a f", f=P),
                    in1=A_rep[:, None, :].to_broadcast([P, nb, P]),
                    op=mybir.AluOpType.mult)
                t2 = sb.tile([P, FB * P], F32, tag="t2")
                nc.vector.tensor_tensor(
                    out=t2[:, :cols].rearrange("p (a f) -> p a f", f=P),
                    in0=t1[:, :cols].rearrange("p (a f) -> p a f", f=P),
                    in1=B_rep[:, None, :].to_broadcast([P, nb, P]),
                    op=mybir.AluOpType.add)
                r = sb.tile([P, FB * P], F32, tag="r")
                nc.scalar.activation(r[:, :cols], t2[:, :cols],
                                     mybir.ActivationFunctionType.Relu)
                hr = sb.tile([P, FB * P], F32, tag="hr")
                nc.sync.dma_start(
                    out=hr[:, :cols].rearrange("p (a f) -> p a f", f=P),
                    in_=hres[c0:c0 + cols, :].rearrange(
                        "(a p) f -> p a f", p=P))
                o = sb.tile([P, FB * P], F32, tag="o")
                nc.vector.tensor_tensor(out=o[:, :cols], in0=r[:, :cols],
                                        in1=hr[:, :cols],
                                        op=mybir.AluOpType.add)
                nc.sync.dma_start(
                    out=out[c0:c0 + cols, :].rearrange("(a p) f -> p a f",
                                                       p=P),
                    in_=o[:, :cols].rearrange("p (a f) -> p a f", f=P))
                wb += nb
            w = W - 1
            t1l = sb.tile([P, P], F32, tag="t1l")
            nc.vector.tensor_tensor(out=t1l[:],
                                    in0=outpre[:, w * P:(w + 1) * P],
                                    in1=A_rep[:], op=mybir.AluOpType.mult)
            t2l = sb.tile([P, P], F32, tag="t2l")
            nc.vector.tensor_tensor(out=t2l[:], in0=t1l[:], in1=B_rep[:],
                                    op=mybir.AluOpType.add)
            rl = sb.tile([P, P], F32, tag="rl")
            nc.scalar.activation(rl[:], t2l[:],
                                 mybir.ActivationFunctionType.Relu)
            hrl = sb.tile([P, P], F32, tag="hrl")
            nc.sync.dma_start(out=hrl[:], in_=hres[w * P:(w + 1) * P, :])
            ol = sb.tile([P, P], F32, tag="ol")
            nc.vector.tensor_tensor(out=ol[:], in0=rl[:], in1=hrl[:],
                                    op=mybir.AluOpType.add)
            nc.sync.dma_start(out=out[w * P:w * P + LASTR, :],
                              in_=ol[:LASTR, :])
    return nc


def host_prepare(h, edge_index, W_l, W_r, bias_l, bias_r, att,
                 bias_out, gamma, beta, n_cores=8):
    N, D = h.shape
    H, C = att.shape
    E = edge_index.shape[1]
    h = np.asarray(h, np.float32)
    ei = np.asarray(edge_index)

    loops = np.arange(N, dtype=np.int64)
    src = np.concatenate([ei[0], loops]).astype(np.int64)
    dst = np.concatenate([ei[1], loops]).astype(np.int64)
    order = np.argsort(dst, kind="stable")
    src_s = src[order].astype(np.int32)
    dst_s = dst[order].astype(np.int32)

    NPC = N // n_cores
    W = math.ceil(NPC / P)
    LPAD = W * P
    bounds = np.searchsorted(dst_s, np.arange(0, N + 1, NPC))

    # per-core degree sort
    percore = []
    kw_all = np.zeros((n_cores, W), np.int64)
    for k in range(n_cores):
        lo, hi = bounds[k], bounds[k + 1]
        s_k = src_s[lo:hi]
        d_k = dst_s[lo:hi] - k * NPC
        deg = np.bincount(d_k, minlength=NPC)
        perm = np.argsort(-deg, kind="stable")   # node order, high-deg first
        dsort = deg[perm]
        dpad = np.concatenate([dsort, np.zeros(LPAD - NPC, np.int64)])
        kw_all[k] = dpad.reshape(W, P).max(axis=1)
        starts = np.concatenate([[0], np.cumsum(deg)])
        percore.append((s_k, deg, perm, starts))
    KW = kw_all.max(axis=0)
    KW = np.maximum(KW, 1)

    cfg = Cfg(N=N, D=D, H=H, E=E, n_cores=n_cores, KW=KW,
              has_bl=bool(np.any(np.asarray(bias_l))),
              has_br=bool(np.any(np.asarray(bias_r))))
    SUMKW = cfg.SUMKW

    # shared tensors
    h_b = np.zeros((cfg.HPAD, P), NPBF16)
    h_b[:N] = h.astype(NPBF16)
    constsA = np.zeros((P, 3), np.float32)
    constsA[:, 0] = np.asarray(att, np.float32).reshape(-1)
    constsA[:, 1] = np.asarray(gamma, np.float32)
    constsA[:, 2] = np.asarray(beta, np.float32)
    constsB = np.zeros((P, 2 * P), np.float32)
    constsB[:, 0:P] = np.asarray(W_l, np.float32)
    constsB[:, P:2 * P] = np.asarray(W_r, np.float32)
    constsB = constsB.astype(NPBF16)
    constsC = np.zeros((64, P), np.float32)
    constsC[0] = np.asarray(bias_l, np.float32)
    constsC[32] = np.asarray(bias_r, np.float32)
    constsC = constsC.astype(NPBF16)

    in_maps = []
    perms = []
    for k in range(n_cores):
        s_k, deg, perm, starts = percore[k]
        perms.append(perm)
        # slot tables [LPAD rows, KW[w] cols per window]
        idx = np.zeros((P, 8 * SUMKW), np.int16)
        msk = np.zeros((P, SUMKW), np.float32)
        par = np.zeros((P, SUMKW), np.float32)
        koff = 0
        for w in range(W):
            Kw = int(KW[w])
            NI = P * Kw
            # srcs[p, j] = j-th edge's src of node perm[w*128+p]
            i16 = np.zeros(NI, np.int32)
            for p in range(P):
                v = w * P + p
                if v >= NPC:
                    # pad row: slot 0 stays idx 0 / par 0 (-> zero half),
                    # unmask it so den >= 1 and no divide-by-zero
                    msk[p, koff] = 1.0
                    continue
                node = perm[v]
                e0, e1 = starts[node], starts[node + 1]
                d = e1 - e0
                if d == 0:
                    msk[p, koff] = 1.0
                    continue
                srcs = s_k[e0:e1].astype(np.int64)
                pos = np.arange(d) * P + p      # slot i = k*128+p
                t = srcs + 1                    # xl[src] at table row src+1
                i16[pos] = t >> 1
                par[p, koff:koff + d] = (t & 1).astype(np.float32)
                msk[p, koff:koff + d] = 1.0
            # wrap: [16, NI/16] then replicate to 128 partitions
            blk = i16.reshape(-1, 16).T.astype(np.int16)
            idx[:, 8 * koff:8 * (koff + Kw)] = np.tile(blk, (8, 1))
            koff += Kw

        gidx = k * NPC + perm                   # global ids, perm order
        hloc_b = np.zeros((LPAD, P), NPBF16)
        hloc_b[:NPC] = h[gidx].astype(NPBF16)
        hres = np.zeros((LPAD, P), np.float32)
        hres[:NPC] = h[gidx]

        in_maps.append({
            "h_b": h_b, "hloc_b": hloc_b, "hres": hres,
            "idxp": idx,
            "maskp": msk.astype(NPBF16),
            "parp": par.astype(NPBF16),
            "constsA": constsA, "constsB": constsB, "constsC": constsC,
        })
    return cfg, in_maps, perms


def kernel(h, edge_index, W_l, W_r, bias_l, bias_r, att,
           bias_out, gamma, beta):
    n_cores = 8
    cfg, in_maps, perms = host_prepare(
        h, edge_index, W_l, W_r, bias_l, bias_r, att, bias_out, gamma, beta,
        n_cores=n_cores)
    nc = bacc.Bacc()
    build_kernel(nc, cfg)
    nc.compile()
    res = run_bass_kernel_spmd(nc, in_maps, core_ids=list(range(n_cores)))
    N = cfg.N
    out_full = np.empty((N, cfg.D), np.float32)
    for k in range(n_cores):
        gidx = k * cfg.NPC + perms[k]
        out_full[gidx] = res.results[k]["out"]
    return out_full
